# revision 9
# baseline (speedup 1.0000x reference)
"""Trainium2 Bass kernel: ComplexGabor1D layer.

reference math (fp32):
    lin = x @ W.T + b                      # [N, 256]
    env = exp(-3600 * lin^2)
    out = stack([env*cos(30*lin), env*sin(30*lin)], -1)   # [N, 256, 2]

Strategy (8 NeuronCores, data parallel over N):
  * Host: transpose each x shard to [256, N_SH] so the contraction dim (i)
    lands on SBUF partitions with fully-contiguous DMA loads; replicate
    W.T ([in, out]) and b on every core.
  * Device, per 512-row "unit": fp32r matmuls (x_shard.T tiles as the
    stationary operand, W.T as the moving operand, bias added via a K=1
    rank-1 matmul) -> lin in PSUM; ACT computes sin/cos straight from PSUM
    into the interleaved output tile (real at even, imag at odd offsets);
    square on ACT or DVE (split to balance engines); ACT exp; DVE multiplies
    the envelope into both strided halves in place; DMA out 1 MiB per unit.
  * ACT activation tables: sin and exp live in different table sets
    (~2.7us per switch), so units are processed in groups: all trig work
    for a group first, then all exp work -> 2 switches per group.
  * cos(t) is computed as sin(t + pi/2).  The argument exceeds the Sin
    LUT's [-pi, pi] window only where |30*lin| > pi/2, i.e. where the
    Gaussian envelope is < 5.2e-5, so the hardware clamp there is
    numerically invisible at the output (abs err <= ~1e-4 of absmax 1.0).
"""

import math

import numpy as np

import concourse.bacc as bacc
import concourse.bass as bass
import concourse.mybir as mybir
import concourse.tile as tile
from concourse.bass_utils import run_bass_kernel_spmd

N_TOTAL = 262144
IN_F = 256
OUT_F = 256
N_CORES = 8
N_SH = N_TOTAL // N_CORES  # 32768 rows per core

CHUNK = 128  # rows per matmul (PSUM partition dim)
CH_PER_UNIT = 4  # chunks per unit -> 512 rows, F=1024 elementwise ops
GROUP_UNITS = 11  # units per ACT-table-set group

OMEGA = 30.0
NEG_SCALE2 = -3600.0  # -(60^2)

F32 = mybir.dt.float32
F32R = mybir.dt.float32r
BF16 = mybir.dt.bfloat16

_BUILD_CACHE = {}


def _build(n_sh, ch_per_unit, group_units):
    """Build the single-core Bass program (SPMD across cores via in_maps)."""
    key = (n_sh, ch_per_unit, group_units)
    if key in _BUILD_CACHE:
        return _BUILD_CACHE[key]

    rows_per_unit = CHUNK * ch_per_unit
    assert n_sh % rows_per_unit == 0
    n_units = n_sh // rows_per_unit

    nc = bacc.Bacc("TRN2", target_bir_lowering=False, debug=False)

    xt = nc.dram_tensor("xt", [IN_F, n_sh], F32R, kind="ExternalInput").ap()
    wt = nc.dram_tensor("wt", [IN_F, OUT_F], F32R, kind="ExternalInput").ap()
    bias = nc.dram_tensor("bias", [2, OUT_F], BF16, kind="ExternalInput").ap()
    ones_in = nc.dram_tensor("ones", [2, CHUNK], BF16, kind="ExternalInput").ap()
    out = nc.dram_tensor("out", [n_sh, 2 * OUT_F], F32, kind="ExternalOutput").ap()

    # [i, n] -> [p, ci, n] with i = ci*128 + p
    xt_r = xt.rearrange("(ci p) n -> p ci n", p=CHUNK)
    # [i, o] -> [p, ci, o]
    wt_r = wt.rearrange("(ci p) o -> p ci o", p=CHUNK)
    # row n = u*rows_per_unit + c*128 + p
    out_r = out.rearrange("(u c p) f -> u p c f", p=CHUNK, c=ch_per_unit)

    with tile.TileContext(nc) as tc:
        with (
            tc.tile_pool(name="consts", bufs=1) as consts,
            tc.tile_pool(name="xt", bufs=6) as xt_pool,
            tc.tile_pool(name="linsb", bufs=group_units + 1) as linsb_pool,
            tc.tile_pool(name="sq", bufs=4) as sq_pool,
            tc.tile_pool(name="outp", bufs=group_units + 1) as out_pool,
            tc.tile_pool(name="lin", bufs=4, space="PSUM") as psum_pool,
        ):
            wt_sb = consts.tile([CHUNK, IN_F // CHUNK, OUT_F], F32R)
            nc.sync.dma_start(wt_sb[:], wt_r[:])
            b_sb = consts.tile([2, OUT_F], BF16)
            nc.sync.dma_start(b_sb[:], bias[:])
            ones = consts.tile([2, CHUNK], BF16)
            nc.sync.dma_start(ones[:], ones_in[:])
            zero_b = consts.tile([CHUNK, 1], F32)
            nc.vector.memset(zero_b[:], 0.0)
            pio2_b = consts.tile([CHUNK, 1], F32)
            nc.vector.memset(pio2_b[:], math.pi / 2)

            prev_act = [None]

            def act_chain(inst):
                # Pin the ACT engine's instruction order to emission order so
                # the scheduler cannot interleave exp into the sin stream
                # (each such jump costs two ~1.3us ACT table loads).
                if prev_act[0] is not None:
                    tile.add_dep_helper(inst.ins, prev_act[0], sync=False,
                                        reason="act table-set order")
                prev_act[0] = inst.ins

            n_groups = (n_units + group_units - 1) // group_units
            for g in range(n_groups):
                units = range(g * group_units, min((g + 1) * group_units, n_units))
                staged = []

                # ---- trig phase (sin table set resident) ----
                for u in units:
                    n0 = u * rows_per_unit
                    xt_t = xt_pool.tile([CHUNK, IN_F // CHUNK, rows_per_unit], F32R)
                    nc.sync.dma_start(xt_t[:], xt_r[:, :, n0 : n0 + rows_per_unit])

                    lin = psum_pool.tile([CHUNK, ch_per_unit, OUT_F], F32)
                    for c in range(ch_per_unit):
                        nc.tensor.matmul(
                            lin[:, c, :],
                            xt_t[:, 0, c * CHUNK : (c + 1) * CHUNK],
                            wt_sb[:, 0, :],
                            start=True,
                            stop=False,
                        )
                        nc.tensor.matmul(
                            lin[:, c, :],
                            xt_t[:, 1, c * CHUNK : (c + 1) * CHUNK],
                            wt_sb[:, 1, :],
                            start=False,
                            stop=False,
                        )
                        # rank-1 bias add: ones.T @ b
                        nc.tensor.matmul(
                            lin[:, c, :],
                            ones[:],
                            b_sb[:],
                            start=False,
                            stop=True,
                        )

                    # Drain PSUM via an unchained DVE copy so the PE is never
                    # gated on the ACT table-set phase order.
                    lin_sb = linsb_pool.tile([CHUNK, ch_per_unit, OUT_F], F32)
                    nc.vector.tensor_copy(lin_sb[:], lin[:])

                    out_t = out_pool.tile([CHUNK, ch_per_unit, 2 * OUT_F], F32)
                    out4 = out_t[:].rearrange("p c (o two) -> p c o two", two=2)
                    # imag = sin(30*lin), real = cos = sin(30*lin + pi/2)
                    act_chain(nc.scalar.activation(
                        out4[:, :, :, 1],
                        lin_sb[:],
                        mybir.ActivationFunctionType.Sin,
                        bias=zero_b[:],
                        scale=OMEGA,
                    ))
                    act_chain(nc.scalar.activation(
                        out4[:, :, :, 0],
                        lin_sb[:],
                        mybir.ActivationFunctionType.Sin,
                        bias=pio2_b[:],
                        scale=OMEGA,
                    ))
                    staged.append((u, out_t, lin_sb))

                # ---- exp phase (exp table set resident) ----
                for u, out_t, lin_sb in staged:
                    sq = sq_pool.tile([CHUNK, ch_per_unit, OUT_F], F32)
                    if u % 16 < 7:
                        # ~44% of squares on ACT for engine balance
                        act_chain(nc.scalar.activation(
                            sq[:],
                            lin_sb[:],
                            mybir.ActivationFunctionType.Square,
                            bias=zero_b[:],
                            scale=1.0,
                        ))
                    else:
                        nc.vector.tensor_mul(sq[:], lin_sb[:], lin_sb[:])
                    act_chain(nc.scalar.activation(
                        sq[:],
                        sq[:],
                        mybir.ActivationFunctionType.Exp,
                        bias=zero_b[:],
                        scale=NEG_SCALE2,
                    ))
                    out4 = out_t[:].rearrange("p c (o two) -> p c o two", two=2)
                    nc.vector.tensor_mul(out4[:, :, :, 0], out4[:, :, :, 0], sq[:])
                    nc.vector.tensor_mul(out4[:, :, :, 1], out4[:, :, :, 1], sq[:])
                    # SWDGE so output stores don't head-of-line block input loads
                    nc.gpsimd.dma_start(out_r[u], out_t[:])

    nc.compile()
    _BUILD_CACHE[key] = nc
    return nc


def run_sharded(x, W, b, trace=False, n_sh=N_SH, ch_per_unit=CH_PER_UNIT,
                group_units=GROUP_UNITS):
    """Shard inputs over the 8 cores, run the Bass kernel, gather output."""
    x = np.ascontiguousarray(x, dtype=np.float32)
    W = np.ascontiguousarray(W, dtype=np.float32)
    b = np.ascontiguousarray(b, dtype=np.float32)
    n = x.shape[0]
    assert n == n_sh * N_CORES and x.shape[1] == IN_F

    nc = _build(n_sh, ch_per_unit, group_units)

    import ml_dtypes

    wt_np = np.ascontiguousarray(W.T)  # [in, out]
    bh = b.astype(ml_dtypes.bfloat16)
    bl = (b - bh.astype(np.float32)).astype(ml_dtypes.bfloat16)
    b_np = np.ascontiguousarray(np.stack([bh, bl], axis=0))  # [2, OUT_F] bf16
    ones_np = np.ones((2, CHUNK), dtype=ml_dtypes.bfloat16)
    in_maps = []
    for s in range(N_CORES):
        xt_np = np.ascontiguousarray(x[s * n_sh : (s + 1) * n_sh].T)  # [in, n_sh]
        in_maps.append({"xt": xt_np, "wt": wt_np, "bias": b_np, "ones": ones_np})

    res = run_bass_kernel_spmd(nc, in_maps, list(range(N_CORES)), trace=trace)
    shards = [
        res.results[s]["out"].reshape(n_sh, OUT_F, 2) for s in range(N_CORES)
    ]
    return np.concatenate(shards, axis=0), res


def kernel(x, W, b):
    out, _ = run_sharded(x, W, b)
    return out


# revision 10
# speedup vs baseline: 1.1699x; 1.1699x over previous
"""Trainium2 Bass kernel: ComplexGabor1D layer.

reference math (fp32):
    lin = x @ W.T + b                      # [N, 256]
    env = exp(-3600 * lin^2)
    out = stack([env*cos(30*lin), env*sin(30*lin)], -1)   # [N, 256, 2]

Strategy (8 NeuronCores, data parallel over N):
  * Host: transpose each x shard to [256, N_SH] so the contraction dim (i)
    lands on SBUF partitions with fully-contiguous DMA loads; replicate
    W.T ([in, out]) and b on every core.
  * Device, per 512-row "unit": fp32r matmuls (x_shard.T tiles as the
    stationary operand, W.T as the moving operand, bias added via a K=1
    rank-1 matmul) -> lin in PSUM; ACT computes sin/cos straight from PSUM
    into the interleaved output tile (real at even, imag at odd offsets);
    square on ACT or DVE (split to balance engines); ACT exp; DVE multiplies
    the envelope into both strided halves in place; DMA out 1 MiB per unit.
  * ACT activation tables: sin and exp live in different table sets
    (~2.7us per switch), so units are processed in groups: all trig work
    for a group first, then all exp work -> 2 switches per group.
  * cos(t) is computed as sin(t + pi/2).  The argument exceeds the Sin
    LUT's [-pi, pi] window only where |30*lin| > pi/2, i.e. where the
    Gaussian envelope is < 5.2e-5, so the hardware clamp there is
    numerically invisible at the output (abs err <= ~1e-4 of absmax 1.0).
"""

import math

import numpy as np

import concourse.bacc as bacc
import concourse.bass as bass
import concourse.mybir as mybir
import concourse.tile as tile
from concourse.bass_utils import run_bass_kernel_spmd

N_TOTAL = 262144
IN_F = 256
OUT_F = 256
N_CORES = 8
N_SH = N_TOTAL // N_CORES  # 32768 rows per core

CHUNK = 128  # rows per matmul (PSUM partition dim)
CH_PER_UNIT = 4  # chunks per unit -> 512 rows, F=1024 elementwise ops
GROUP_UNITS = 11  # units per ACT-table-set group

OMEGA = 30.0
NEG_SCALE2 = -3600.0  # -(60^2)

F32 = mybir.dt.float32
F32R = mybir.dt.float32r
BF16 = mybir.dt.bfloat16

_BUILD_CACHE = {}


def _build(n_sh, ch_per_unit, group_units):
    """Build the single-core Bass program (SPMD across cores via in_maps)."""
    key = (n_sh, ch_per_unit, group_units)
    if key in _BUILD_CACHE:
        return _BUILD_CACHE[key]

    rows_per_unit = CHUNK * ch_per_unit
    assert n_sh % rows_per_unit == 0
    n_units = n_sh // rows_per_unit

    nc = bacc.Bacc("TRN2", target_bir_lowering=False, debug=False)

    xt = nc.dram_tensor("xt", [IN_F, n_sh], F32R, kind="ExternalInput").ap()
    wt = nc.dram_tensor("wt", [IN_F, OUT_F], F32R, kind="ExternalInput").ap()
    bias = nc.dram_tensor(
        "bias", [CHUNK, ch_per_unit * OUT_F], F32, kind="ExternalInput"
    ).ap()
    out = nc.dram_tensor("out", [n_sh, 2 * OUT_F], F32, kind="ExternalOutput").ap()

    # [i, n] -> [p, ci, n] with i = ci*128 + p
    xt_r = xt.rearrange("(ci p) n -> p ci n", p=CHUNK)
    # [i, o] -> [p, ci, o]
    wt_r = wt.rearrange("(ci p) o -> p ci o", p=CHUNK)
    # row n = u*rows_per_unit + c*128 + p
    out_r = out.rearrange("(u c p) f -> u p c f", p=CHUNK, c=ch_per_unit)

    with tile.TileContext(nc) as tc:
        with (
            tc.tile_pool(name="consts", bufs=1) as consts,
            tc.tile_pool(name="xt", bufs=6) as xt_pool,
            tc.tile_pool(name="linsb", bufs=group_units + 1) as linsb_pool,
            tc.tile_pool(name="sq", bufs=4) as sq_pool,
            tc.tile_pool(name="outp", bufs=group_units + 1) as out_pool,
            tc.tile_pool(name="lin", bufs=4, space="PSUM") as psum_pool,
        ):
            wt_sb = consts.tile([CHUNK, IN_F // CHUNK, OUT_F], F32R)
            nc.sync.dma_start(wt_sb[:], wt_r[:])
            # bias broadcast across all 128 partitions, tiled x4 along free
            b_sb = consts.tile([CHUNK, ch_per_unit, OUT_F], F32)
            nc.sync.dma_start(
                b_sb[:], bias.rearrange("p (c o) -> p c o", c=ch_per_unit)
            )
            zero_b = consts.tile([CHUNK, 1], F32)
            nc.vector.memset(zero_b[:], 0.0)
            pio2_b = consts.tile([CHUNK, 1], F32)
            nc.vector.memset(pio2_b[:], math.pi / 2)

            prev_act = [None]

            def act_chain(inst):
                # Pin the ACT engine's instruction order to emission order so
                # the scheduler cannot interleave exp into the sin stream
                # (each such jump costs two ~1.3us ACT table loads).
                if prev_act[0] is not None:
                    tile.add_dep_helper(inst.ins, prev_act[0], sync=False,
                                        reason="act table-set order")
                prev_act[0] = inst.ins

            n_groups = (n_units + group_units - 1) // group_units
            for g in range(n_groups):
                units = range(g * group_units, min((g + 1) * group_units, n_units))
                staged = []

                # ---- trig phase (sin table set resident) ----
                for u in units:
                    n0 = u * rows_per_unit
                    xt_t = xt_pool.tile([CHUNK, IN_F // CHUNK, rows_per_unit], F32R)
                    nc.sync.dma_start(xt_t[:], xt_r[:, :, n0 : n0 + rows_per_unit])

                    lin = psum_pool.tile([CHUNK, ch_per_unit, OUT_F], F32)
                    for c in range(ch_per_unit):
                        nc.tensor.matmul(
                            lin[:, c, :],
                            xt_t[:, 0, c * CHUNK : (c + 1) * CHUNK],
                            wt_sb[:, 0, :],
                            start=True,
                            stop=False,
                        )
                        nc.tensor.matmul(
                            lin[:, c, :],
                            xt_t[:, 1, c * CHUNK : (c + 1) * CHUNK],
                            wt_sb[:, 1, :],
                            start=False,
                            stop=True,
                        )

                    # Drain PSUM via an unchained DVE copy so the PE is never
                    # gated on the ACT table-set phase order.
                    lin_sb = linsb_pool.tile([CHUNK, ch_per_unit, OUT_F], F32)
                    nc.vector.scalar_tensor_tensor(
                        lin_sb[:],
                        lin[:],
                        1.0,
                        b_sb[:],
                        op0=mybir.AluOpType.mult,
                        op1=mybir.AluOpType.add,
                    )

                    out_t = out_pool.tile([CHUNK, ch_per_unit, 2 * OUT_F], F32)
                    out4 = out_t[:].rearrange("p c (o two) -> p c o two", two=2)
                    # imag = sin(30*lin), real = cos = sin(30*lin + pi/2)
                    act_chain(nc.scalar.activation(
                        out4[:, :, :, 1],
                        lin_sb[:],
                        mybir.ActivationFunctionType.Sin,
                        bias=zero_b[:],
                        scale=OMEGA,
                    ))
                    act_chain(nc.scalar.activation(
                        out4[:, :, :, 0],
                        lin_sb[:],
                        mybir.ActivationFunctionType.Sin,
                        bias=pio2_b[:],
                        scale=OMEGA,
                    ))
                    staged.append((u, out_t, lin_sb))

                # ---- exp phase (exp table set resident) ----
                for u, out_t, lin_sb in staged:
                    sq = sq_pool.tile([CHUNK, ch_per_unit, OUT_F], F32)
                    if u % 16 < 7:
                        # ~44% of squares on ACT for engine balance
                        act_chain(nc.scalar.activation(
                            sq[:],
                            lin_sb[:],
                            mybir.ActivationFunctionType.Square,
                            bias=zero_b[:],
                            scale=1.0,
                        ))
                    else:
                        nc.vector.tensor_mul(sq[:], lin_sb[:], lin_sb[:])
                    act_chain(nc.scalar.activation(
                        sq[:],
                        sq[:],
                        mybir.ActivationFunctionType.Exp,
                        bias=zero_b[:],
                        scale=NEG_SCALE2,
                    ))
                    out4 = out_t[:].rearrange("p c (o two) -> p c o two", two=2)
                    nc.vector.tensor_mul(out4[:, :, :, 0], out4[:, :, :, 0], sq[:])
                    nc.vector.tensor_mul(out4[:, :, :, 1], out4[:, :, :, 1], sq[:])
                    # SWDGE so output stores don't head-of-line block input loads
                    nc.gpsimd.dma_start(out_r[u], out_t[:])

    nc.compile()
    _BUILD_CACHE[key] = nc
    return nc


def run_sharded(x, W, b, trace=False, n_sh=N_SH, ch_per_unit=CH_PER_UNIT,
                group_units=GROUP_UNITS):
    """Shard inputs over the 8 cores, run the Bass kernel, gather output."""
    x = np.ascontiguousarray(x, dtype=np.float32)
    W = np.ascontiguousarray(W, dtype=np.float32)
    b = np.ascontiguousarray(b, dtype=np.float32)
    n = x.shape[0]
    assert n == n_sh * N_CORES and x.shape[1] == IN_F

    nc = _build(n_sh, ch_per_unit, group_units)

    wt_np = np.ascontiguousarray(W.T)  # [in, out]
    b_np = np.ascontiguousarray(
        np.broadcast_to(
            np.tile(b, ch_per_unit)[None, :], (CHUNK, ch_per_unit * OUT_F)
        )
    )
    in_maps = []
    for s in range(N_CORES):
        xt_np = np.ascontiguousarray(x[s * n_sh : (s + 1) * n_sh].T)  # [in, n_sh]
        in_maps.append({"xt": xt_np, "wt": wt_np, "bias": b_np})

    res = run_bass_kernel_spmd(nc, in_maps, list(range(N_CORES)), trace=trace)
    shards = [
        res.results[s]["out"].reshape(n_sh, OUT_F, 2) for s in range(N_CORES)
    ]
    return np.concatenate(shards, axis=0), res


def kernel(x, W, b):
    out, _ = run_sharded(x, W, b)
    return out
